# revision 5
# baseline (speedup 1.0000x reference)
"""CrossPhaseRoutingLayer Trainium2 kernel.

Full inputs -> full output. Data-parallel over the fused B*C=512 sequence axis
across 8 NeuronCores (64 sequences each). Per core, sequences are processed in
groups of G=4 (T = G*96 = 384 token columns per group).

Algebraic restructuring (host-side, weight-only folds, validated to ~6e-6):
  - Sender attention q = router @ Wq_s + bq_s is input-independent, so sender
    scores fold into one matrix: scores^T = M_score^T @ x^T + c_score, where
    M_score[d,(h,r)] = Wk_s[d,h-slice] . q_s[r,h-slice] / sqrt(E).
  - The sender value/output path runs in "mix first, project later" order:
    T_mix = A1 @ x (per head/router), then per-head Wv_s slice, then Wo_s.
    Sender biases collapse: c_send = bv_s @ Wo_s + bo_s.
  - Receiver: scale folds into Wq_r/bq_r; bv_r folds into c_recv = bv_r@Wo_r+bo_r.
  - Softmaxes skip max-subtraction (scores provably tiny: |s| < 0.1).

On-chip layout: activations live transposed (x^T: [D-chunk(128 part), token])
for all D-contraction matmuls; token-partition tiles where per-token free-dim
reductions (softmax) or token-contraction (A1 @ x) are needed; PE transposes
convert. Big matmuls (N>=256) run as float32r (~2.7x fp32 rate, rel err ~1e-4).
LayerNorm over the partitioned D axis uses ones-matmul reductions and a
[K=1] broadcast matmul.
"""
import numpy as np

import concourse.bacc as bacc
import concourse.bass as bass
import concourse.mybir as mybir
import concourse.tile as tile
from concourse.bass_utils import run_bass_kernel_spmd
from concourse.masks import make_identity

FP = mybir.dt.float32
FPR = mybir.dt.float32r
AX = mybir.AxisListType
OP = mybir.AluOpType
ACTF = mybir.ActivationFunctionType

B, C, L, D = 16, 32, 96, 512
R, H = 8, 4
E = D // H            # 128
HR = H * R            # 32
DC = D // 128         # 4 D-chunks
OC = (4 * D) // 128   # 16 MLP hidden chunks
EPS = 1e-5
N_CORES = 8
G = 4                 # sequences per group
T = G * L             # 384 token columns per group

W_NAMES = ["Msc", "Wv_s", "Wo_s", "Wq_r", "Wk_r", "Wv_r", "Wo_r", "W1", "W2"]
V_NAMES = ["c_score", "c_send", "c_recv", "bq_r", "bk_r", "b1", "b2",
           "ln1_g", "ln1_b", "ln2_g", "ln2_b"]


def build_core_kernel(n_seq: int):
    """Bass program for one core processing n_seq sequences."""
    assert n_seq % G == 0
    n_groups = n_seq // G
    nc = bacc.Bacc(None)

    z = nc.declare_dram_parameter("z", [n_seq * L, D], FP, isOutput=False)
    out = nc.declare_dram_parameter("out", [n_seq * L, D], FP, isOutput=True)
    wd = {}
    for name, shape in [("Msc", [D, HR]), ("Wv_s", [D, D]), ("Wo_s", [D, D]),
                        ("Wq_r", [D, D]), ("Wk_r", [D, D]), ("Wv_r", [D, D]),
                        ("Wo_r", [D, D]), ("W1", [D, 4 * D]), ("W2", [4 * D, D])]:
        wd[name] = nc.declare_dram_parameter(name, shape, FPR, isOutput=False)
    for name, n in [("c_score", HR), ("c_send", D), ("c_recv", D), ("bq_r", D),
                    ("bk_r", D), ("b1", 4 * D), ("b2", D), ("ln1_g", D),
                    ("ln1_b", D), ("ln2_g", D), ("ln2_b", D)]:
        wd[name] = nc.declare_dram_parameter(name, [n], FP, isOutput=False)

    with tile.TileContext(nc) as tc:
        with tc.tile_pool(name="wpool", bufs=1) as wp, \
             tc.tile_pool(name="xin", bufs=2) as px, \
             tc.tile_pool(name="act1", bufs=1) as pa, \
             tc.tile_pool(name="sm", bufs=2) as psm, \
             tc.tile_pool(name="micro", bufs=3) as pmi, \
             tc.tile_pool(name="big3", bufs=3) as pb3, \
             tc.tile_pool(name="otok", bufs=2) as po, \
             tc.tile_pool(name="ps", bufs=1, space="PSUM") as ps:

            # ---------------- resident weights / constants -----------------
            w = {}
            w["Msc"] = wp.tile([128, DC, HR], FPR, name="w_Msc")
            for name in ["Wv_s", "Wo_s", "Wq_r", "Wk_r", "Wv_r", "Wo_r"]:
                w[name] = wp.tile([128, DC, D], FPR, name=f"w_{name}")
            w["W1"] = wp.tile([128, DC, 4 * D], FPR, name="w_W1")
            w["W2"] = wp.tile([128, OC, D], FPR, name="w_W2")
            for name in W_NAMES:
                nc.sync.dma_start(
                    out=w[name],
                    in_=wd[name].rearrange("(c p) x -> p c x", p=128))
            w["c_score"] = wp.tile([HR, 1], FP, name="w_c_score")
            nc.sync.dma_start(out=w["c_score"],
                              in_=wd["c_score"].rearrange("(p o) -> p o", o=1))
            for name in ["c_send", "c_recv", "bq_r", "bk_r", "b2",
                         "ln1_g", "ln1_b", "ln2_g", "ln2_b"]:
                w[name] = wp.tile([128, DC], FP, name=f"w_{name}")
                nc.sync.dma_start(out=w[name],
                                  in_=wd[name].rearrange("(c p) -> p c", p=128))
            w["b1"] = wp.tile([128, OC], FP, name="w_b1")
            nc.sync.dma_start(out=w["b1"],
                              in_=wd["b1"].rearrange("(c p) -> p c", p=128))

            ident = wp.tile([128, 128], FP, name="ident")
            make_identity(nc, ident)
            ones_f = wp.tile([128, 1], FP, name="ones_f")
            nc.vector.memset(ones_f, 1.0)
            ones_r = wp.tile([128, 1], FPR, name="ones_r")
            nc.scalar.copy(out=ones_r, in_=ones_f)
            ones_b = wp.tile([1, 128], FP, name="ones_b")
            nc.vector.memset(ones_b, 1.0)
            eps_t = wp.tile([1, 1], FP, name="eps_t")
            nc.vector.memset(eps_t, EPS)

            for gi in range(n_groups):
                group_body(nc, tc, w, ident, ones_r, ones_b, eps_t,
                           z, out, gi,
                           px, pa, psm, pmi, pb3, po, ps)
    nc.finalize()
    return nc


def layernorm_T(nc, w, ones_r, ones_b, eps_t, pmi, pb3, ps,
                s_T, out_tile, g_name, b_name, out_dtype, tag):
    """LN over the partition-split D axis of s_T [128, DC, T] -> out_tile."""
    mean_ps = ps.tile([1, T], FP, name=f"mean_ps{tag}", tag="big", bufs=2)
    for k in range(DC):
        nc.tensor.matmul(out=mean_ps, lhsT=ones_r, rhs=s_T[:, k, :],
                         start=(k == 0), stop=(k == DC - 1))
    msc = pmi.tile([1, T], FP, name=f"msc{tag}", tag="micro")
    nc.scalar.activation(out=msc, in_=mean_ps, func=ACTF.Copy, scale=1.0 / D)

    ss_ps = ps.tile([1, T], FP, name=f"ss_ps{tag}", tag="big", bufs=2)
    for k in range(DC):
        sq = pb3.tile([128, T], FPR, name=f"sq{tag}", tag="sq")
        nc.vector.tensor_mul(out=sq, in0=s_T[:, k, :].bitcast(FP),
                             in1=s_T[:, k, :].bitcast(FP))
        nc.tensor.matmul(out=ss_ps, lhsT=ones_r, rhs=sq,
                         start=(k == 0), stop=(k == DC - 1))

    msc2 = pmi.tile([1, T], FP, name=f"msc2{tag}", tag="micro")
    nc.vector.tensor_mul(out=msc2, in0=msc, in1=msc)
    var_s = pmi.tile([1, T], FP, name=f"var{tag}", tag="micro")
    nc.vector.scalar_tensor_tensor(out=var_s, in0=ss_ps, scalar=1.0 / D,
                                   in1=msc2, op0=OP.mult, op1=OP.subtract)
    srt = pmi.tile([1, T], FP, name=f"srt{tag}", tag="micro")
    nc.scalar.activation(out=srt, in_=var_s, func=ACTF.Sqrt, bias=eps_t)
    rstd = pmi.tile([1, T], FP, name=f"rstd{tag}", tag="micro")
    nc.vector.reciprocal(out=rstd, in_=srt)
    mr = pmi.tile([1, T], FP, name=f"mr{tag}", tag="micro")
    nc.vector.tensor_mul(out=mr, in0=msc, in1=rstd)

    rstdB = ps.tile([128, T], FP, name=f"rstdB{tag}", tag="big", bufs=2)
    nc.tensor.matmul(out=rstdB, lhsT=ones_b, rhs=rstd, start=True, stop=True)
    mrB = ps.tile([128, T], FP, name=f"mrB{tag}", tag="big", bufs=2)
    nc.tensor.matmul(out=mrB, lhsT=ones_b, rhs=mr, start=True, stop=True)

    for k in range(DC):
        t1 = pb3.tile([128, T], FP, name=f"t1{tag}", tag="lnt")
        nc.vector.tensor_mul(out=t1, in0=s_T[:, k, :].bitcast(FP), in1=rstdB)
        nc.vector.tensor_sub(out=t1, in0=t1, in1=mrB)
        nc.vector.tensor_scalar(out=out_tile[:, k, :],
                                in0=t1,
                                scalar1=w[g_name][:, k:k + 1], op0=OP.mult,
                                scalar2=w[b_name][:, k:k + 1], op1=OP.add)


def group_body(nc, tc, w, ident, ones_r, ones_b, eps_t, z, out, gi,
               px, pa, psm, pmi, pb3, po, ps):
    r0 = gi * T   # first DRAM row of the group

    # ---- load x (token-partition) and build x^T ----
    x_tok = px.tile([L, G, D], FP, name="x_tok")
    nc.sync.dma_start(out=x_tok,
                      in_=z[r0:r0 + T, :].rearrange("(g l) d -> l g d", g=G))
    xT = pa.tile([128, DC, T], FPR, name="xT")
    for g in range(G):
        for dc in range(DC):
            pt = ps.tile([128, L], FP, name="pt_x", tag="sp", bufs=2)
            nc.tensor.transpose(out=pt, in_=x_tok[:, g, dc * 128:(dc + 1) * 128],
                                identity=ident[:L, :L])
            nc.scalar.copy(out=xT[:, dc, g * L:(g + 1) * L], in_=pt)
    xTr = xT  # FPR view; read with .bitcast(FP)

    # ---- sender scores^T [HR, T] and softmax over tokens ----
    sc_ps = ps.tile([HR, T], FP, name="sc_ps", tag="big", bufs=2)
    for k in range(DC):
        nc.tensor.matmul(out=sc_ps, lhsT=w["Msc"][:, k, :], rhs=xTr[:, k, :],
                         start=(k == 0), stop=(k == DC - 1))
    e1 = psm.tile([HR, T], FP, name="e1")
    nc.scalar.activation(out=e1, in_=sc_ps, func=ACTF.Exp, bias=w["c_score"])
    s1sum = psm.tile([HR, G], FP, name="s1sum")
    nc.vector.tensor_reduce(out=s1sum, in_=e1.rearrange("p (g l) -> p g l", g=G),
                            axis=AX.X, op=OP.add)
    r1 = psm.tile([HR, G], FP, name="r1")
    nc.vector.reciprocal(out=r1, in_=s1sum)

    # A1^T per sequence (token-partition), un/normalized handling:
    # normalize in [HR, L] layout then transpose to [L, HR].
    a1t = []
    for g in range(G):
        a1n = psm.tile([HR, L], FP, name=f"a1n{g}", tag="a1n", bufs=2)
        nc.vector.tensor_scalar_mul(out=a1n, in0=e1[:, g * L:(g + 1) * L],
                                    scalar1=r1[:, g:g + 1])
        a1p = ps.tile([L, HR], FP, name="a1p", tag="sp", bufs=2)
        nc.tensor.transpose(out=a1p, in_=a1n, identity=ident[:HR, :HR])
        a1s = psm.tile([L, HR], FP, name=f"a1s{g}", tag="a1s", bufs=4)
        nc.scalar.copy(out=a1s, in_=a1p)
        a1t.append(a1s)

    # ---- T_mix^T [(dc), (g, hr)] = x_chunk.T @ A1^T  (contract tokens) ----
    tm_ps = ps.tile([128, DC, G, HR], FP, name="tm_ps", tag="sp", bufs=2)
    for g in range(G):
        for dc in range(DC):
            nc.tensor.matmul(out=tm_ps[:, dc, g, :],
                             lhsT=x_tok[:, g, dc * 128:(dc + 1) * 128],
                             rhs=a1t[g], start=True, stop=True)
    TmT = pa.tile([128, DC, G, HR], FPR, name="TmT")
    nc.scalar.copy(out=TmT, in_=tm_ps)

    # ---- out_cat^T chunk h = Wv_s_h^T @ Tm_h^T   [128,(g,r)] ----
    oc_ps = ps.tile([128, H, G, R], FP, name="oc_ps", tag="sp", bufs=2)
    for h in range(H):
        for k in range(DC):
            nc.tensor.matmul(out=oc_ps[:, h, :, :],
                             lhsT=w["Wv_s"][:, k, h * E:(h + 1) * E],
                             rhs=TmT[:, k, :, h * R:(h + 1) * R],
                             start=(k == 0), stop=(k == DC - 1))
    Oc = pa.tile([128, H, G, R], FPR, name="Oc")
    nc.scalar.copy(out=Oc, in_=oc_ps)

    # ---- router_buffer^T [(dc), (g, r)] = Wo_s^T @ out_cat^T + c_send ----
    rb_ps = ps.tile([128, DC, G, R], FP, name="rb_ps", tag="sp", bufs=2)
    for dc in range(DC):
        for k in range(DC):
            nc.tensor.matmul(out=rb_ps[:, dc, :, :],
                             lhsT=w["Wo_s"][:, k, dc * 128:(dc + 1) * 128],
                             rhs=Oc[:, k, :, :],
                             start=(k == 0), stop=(k == DC - 1))
    rb = pa.tile([128, DC, G, R], FPR, name="rb")
    for dc in range(DC):
        nc.scalar.activation(out=rb[:, dc, :, :],
                             in_=rb_ps[:, dc, :, :], func=ACTF.Identity,
                             bias=w["c_send"][:, dc:dc + 1])

    # ---- receiver k^T [(dc=head), (g,r)] ----
    kt_ps = ps.tile([128, DC, G, R], FP, name="kt_ps", tag="sp", bufs=2)
    for dc in range(DC):
        for k in range(DC):
            nc.tensor.matmul(out=kt_ps[:, dc, :, :],
                             lhsT=w["Wk_r"][:, k, dc * 128:(dc + 1) * 128],
                             rhs=rb[:, k, :, :],
                             start=(k == 0), stop=(k == DC - 1))
    kT = pa.tile([128, DC, G, R], FPR, name="kT")
    for dc in range(DC):
        nc.scalar.activation(out=kT[:, dc, :, :],
                             in_=kt_ps[:, dc, :, :], func=ACTF.Identity,
                             bias=w["bk_r"][:, dc:dc + 1])

    # ---- receiver v in router-partition layout [8, D] per seq (bias folded) ----
    v_sb = []
    for g in range(G):
        v_ps = ps.tile([R, D], FP, name="v_ps", tag="sp", bufs=2)
        for k in range(DC):
            nc.tensor.matmul(out=v_ps, lhsT=rb[:, k, g, :],
                             rhs=w["Wv_r"][:, k, :],
                             start=(k == 0), stop=(k == DC - 1))
        v_g = psm.tile([R, D], FPR, name=f"v_g{g}", tag="v_g", bufs=4)
        nc.scalar.copy(out=v_g, in_=v_ps)
        v_sb.append(v_g)

    # ---- receiver q^T [(dc), T] (scale+bias pre-folded) ----
    qT = pa.tile([128, DC, T], FPR, name="qT", tag="big_a")
    for dc in range(DC):
        q_ps = ps.tile([128, T], FP, name="q_ps", tag="big", bufs=2)
        for k in range(DC):
            nc.tensor.matmul(out=q_ps, lhsT=w["Wq_r"][:, k, dc * 128:(dc + 1) * 128],
                             rhs=xTr[:, k, :], start=(k == 0), stop=(k == DC - 1))
        nc.scalar.activation(out=qT[:, dc, :], in_=q_ps,
                             func=ACTF.Identity, bias=w["bq_r"][:, dc:dc + 1])

    # ---- receiver scores -> softmax (over R, free dim) -> e2, r2 ----
    aT = pa.tile([128, DC, T], FPR, name="aT", tag="big_b")
    for g in range(G):
        s2_ps = ps.tile([L, H, R], FP, name="s2_ps", tag="sp", bufs=2)
        for h in range(H):
            nc.tensor.matmul(out=s2_ps[:, h, :],
                             lhsT=qT[:, h, g * L:(g + 1) * L],
                             rhs=kT[:, h, g, :], start=True, stop=True)
        e2 = psm.tile([L, H, R], FP, name=f"e2{g}", tag="e2", bufs=2)
        nc.scalar.activation(out=e2, in_=s2_ps, func=ACTF.Exp)
        ssum = psm.tile([L, H], FP, name=f"ssum{g}", tag="ssum", bufs=2)
        nc.vector.tensor_reduce(out=ssum, in_=e2, axis=AX.X, op=OP.add)
        r2 = psm.tile([L, H], FP, name=f"r2{g}", tag="r2", bufs=2)
        nc.vector.reciprocal(out=r2, in_=ssum)

        # e2^T per head [R, L] (PE transpose), rounded for the mix matmul
        e2t = psm.tile([R, H, L], FPR, name=f"e2t{g}", tag="e2t", bufs=2)
        for h in range(H):
            e2p = ps.tile([R, L], FP, name="e2p", tag="sp", bufs=2)
            nc.tensor.transpose(out=e2p, in_=e2[:, h, :], identity=ident[:L, :L])
            nc.scalar.copy(out=e2t[:, h, :], in_=e2p)

        # mix: attn_pre[l, h*E:(h+1)*E] = e2_h @ v_h, then normalize by r2
        ap_ps = ps.tile([L, D], FP, name="ap_ps", tag="big", bufs=2)
        for h in range(H):
            nc.tensor.matmul(out=ap_ps[:, h * E:(h + 1) * E],
                             lhsT=e2t[:, h, :],
                             rhs=v_sb[g][:, h * E:(h + 1) * E],
                             start=True, stop=True)
        apn = psm.tile([L, D], FP, name=f"apn{g}", tag="apn", bufs=2)
        for h in range(H):
            nc.vector.tensor_scalar_mul(out=apn[:, h * E:(h + 1) * E],
                                        in0=ap_ps[:, h * E:(h + 1) * E],
                                        scalar1=r2[:, h:h + 1])
        # transpose attn_pre into aT columns for this sequence
        for dc in range(DC):
            app = ps.tile([128, L], FP, name="app", tag="sp", bufs=2)
            nc.tensor.transpose(out=app, in_=apn[:, dc * 128:(dc + 1) * 128],
                                identity=ident[:L, :L])
            nc.scalar.copy(out=aT[:, dc, g * L:(g + 1) * L], in_=app)

    # ---- attn2^T = Wo_r^T @ attn_pre^T + c_recv; residual; LN1 ----
    s1T = pa.tile([128, DC, T], FPR, name="s1T", tag="big_a")
    for dc in range(DC):
        at2_ps = ps.tile([128, T], FP, name="at2_ps", tag="big", bufs=2)
        for k in range(DC):
            nc.tensor.matmul(out=at2_ps,
                             lhsT=w["Wo_r"][:, k, dc * 128:(dc + 1) * 128],
                             rhs=aT[:, k, :], start=(k == 0), stop=(k == DC - 1))
        nc.vector.scalar_tensor_tensor(out=s1T[:, dc, :],
                                       in0=at2_ps,
                                       scalar=w["c_recv"][:, dc:dc + 1],
                                       in1=xTr[:, dc, :].bitcast(FP),
                                       op0=OP.add, op1=OP.add)
    out1T = pa.tile([128, DC, T], FPR, name="out1T", tag="big_b")
    layernorm_T(nc, w, ones_r, ones_b, eps_t, pmi, pb3, ps,
                s1T, out1T, "ln1_g", "ln1_b", FPR, f"_l1_{gi}")

    # ---- MLP ----
    h2_ps = [ps.tile([128, T], FP, name=f"h2_ps{dc}", tag=f"h2_{dc}", bufs=1)
             for dc in range(DC)]
    for oc in range(OC):
        h1_ps = ps.tile([128, T], FP, name="h1_ps", tag="big", bufs=2)
        for k in range(DC):
            nc.tensor.matmul(out=h1_ps,
                             lhsT=w["W1"][:, k, oc * 128:(oc + 1) * 128],
                             rhs=out1T[:, k, :], start=(k == 0), stop=(k == DC - 1))
        gl = pb3.tile([128, T], FPR, name="gl", tag="gl")
        nc.scalar.activation(out=gl, in_=h1_ps, func=ACTF.Gelu,
                             bias=w["b1"][:, oc:oc + 1])
        for dc in range(DC):
            nc.tensor.matmul(out=h2_ps[dc],
                             lhsT=w["W2"][:, oc, dc * 128:(dc + 1) * 128],
                             rhs=gl, start=(oc == 0), stop=(oc == OC - 1))

    # ---- residual2 + LN2 -> outT (fp32, for output transposes) ----
    s2T = pa.tile([128, DC, T], FPR, name="s2T", tag="big_a")
    for dc in range(DC):
        nc.vector.scalar_tensor_tensor(out=s2T[:, dc, :],
                                       in0=h2_ps[dc],
                                       scalar=w["b2"][:, dc:dc + 1],
                                       in1=out1T[:, dc, :].bitcast(FP),
                                       op0=OP.add, op1=OP.add)
    outT = pa.tile([128, DC, T], FP, name="outT", tag="outT")
    layernorm_T(nc, w, ones_r, ones_b, eps_t, pmi, pb3, ps,
                s2T, outT, "ln2_g", "ln2_b", FP, f"_l2_{gi}")

    # ---- transpose back to token rows and store ----
    out_tok = po.tile([128, T // 128, D], FP, name="out_tok")
    for a in range(T // 128):
        for dc in range(DC):
            op_ps = ps.tile([128, 128], FP, name="op_ps", tag="sp", bufs=2)
            nc.tensor.transpose(out=op_ps,
                                in_=outT[:, dc, a * 128:(a + 1) * 128],
                                identity=ident)
            nc.scalar.copy(out=out_tok[:, a, dc * 128:(dc + 1) * 128], in_=op_ps)
    nc.gpsimd.dma_start(out=out[r0:r0 + T, :].rearrange("(a p) d -> p a d", p=128),
                        in_=out_tok)


def _host_fold(inputs):
    """Host-side weight-only precomputation."""
    f32 = np.float32
    scale = 1.0 / np.sqrt(np.float32(E))
    q_s = (inputs["router"] @ inputs["Wq_s"] + inputs["bq_s"]).astype(f32)
    q_sh = q_s.reshape(R, H, E)
    Wk = inputs["Wk_s"].reshape(D, H, E)
    M_score = (np.einsum("dhe,rhe->dhr", Wk, q_sh).reshape(D, HR) * scale).astype(f32)
    c_score = (np.einsum("he,rhe->hr", inputs["bk_s"].reshape(H, E), q_sh)
               .reshape(HR) * scale).astype(f32)
    c_send = (inputs["bv_s"] @ inputs["Wo_s"] + inputs["bo_s"]).astype(f32)
    c_recv = (inputs["bv_r"] @ inputs["Wo_r"] + inputs["bo_r"]).astype(f32)
    return {
        "Msc": np.ascontiguousarray(M_score),
        "c_score": c_score,
        "c_send": c_send,
        "c_recv": c_recv,
        "Wv_s": np.ascontiguousarray(inputs["Wv_s"].astype(f32)),
        "Wo_s": np.ascontiguousarray(inputs["Wo_s"].astype(f32)),
        "Wq_r": np.ascontiguousarray((inputs["Wq_r"] * scale).astype(f32)),
        "bq_r": (inputs["bq_r"] * scale).astype(f32),
        "Wk_r": np.ascontiguousarray(inputs["Wk_r"].astype(f32)),
        "bk_r": inputs["bk_r"].astype(f32),
        "Wv_r": np.ascontiguousarray(inputs["Wv_r"].astype(f32)),
        "Wo_r": np.ascontiguousarray(inputs["Wo_r"].astype(f32)),
        "W1": np.ascontiguousarray(inputs["W1"].astype(f32)),
        "b1": inputs["b1"].astype(f32),
        "W2": np.ascontiguousarray(inputs["W2"].astype(f32)),
        "b2": inputs["b2"].astype(f32),
        "ln1_g": inputs["ln1_g"].astype(f32),
        "ln1_b": inputs["ln1_b"].astype(f32),
        "ln2_g": inputs["ln2_g"].astype(f32),
        "ln2_b": inputs["ln2_b"].astype(f32),
    }


def kernel(**inputs) -> np.ndarray:
    inputs = {k: np.asarray(v) for k, v in inputs.items()}
    Z = inputs["Z"].astype(np.float32)
    n_seq_total = B * C
    n_seq = n_seq_total // N_CORES
    folded = _host_fold(inputs)

    nc = build_core_kernel(n_seq)
    Zf = Z.reshape(n_seq_total, L, D)
    in_maps = []
    for c in range(N_CORES):
        m = {"z": np.ascontiguousarray(
            Zf[c * n_seq:(c + 1) * n_seq].reshape(n_seq * L, D))}
        m.update(folded)
        in_maps.append(m)
    res = run_bass_kernel_spmd(nc, in_maps, list(range(N_CORES)))
    out = np.empty((n_seq_total, L, D), np.float32)
    for c in range(N_CORES):
        out[c * n_seq:(c + 1) * n_seq] = res.results[c]["out"].reshape(n_seq, L, D)
    return out.reshape(B, C, L, D)


if __name__ == "__main__":
    import reference
    inputs = reference.setup_inputs()
    inputs = {k: np.asarray(v) for k, v in inputs.items()}
    expected = np.asarray(reference.reference(**inputs))
    got = kernel(**inputs)
    err = np.abs(got - expected).max()
    rel = err / np.abs(expected).max()
    print(f"abs err {err:.3e}  absmax-rel {rel:.3e}")
